# revision 42
# baseline (speedup 1.0000x reference)
"""EulerAttention Trainium2 kernel (fp8/bf16-accelerated, restructured).

Per-core sharding: core c in 0..7 -> (batch b = c // 4, query block qb = c % 4,
1024 queries each).  Each core computes K/V (+ feature maps) for its whole
batch, Q features for its query block, then scores/softmax/AV.

Precision scheme:
- Q/K/V projections: e-tile 0 f32r; e-tile 1 and V 3-pass fp8 DoubleRow
  (xh*Wh + xl*Wh + xh*Wl); e-tiles 2-7 2-pass (the inv_wl prescale shrinks
  the W-residual below the feature noise floor).  W is pre-scaled x64 to
  clear the e4m3 subnormal floor and (for Q/K) pre-scaled per-feature by
  inv_wavelength/2pi so the projection PSUM lands directly in (64x) "turns"
  units -- no post-matmul scale/bias ACT; the turns-space MAGIC range
  reduction runs at the x64 scale (MAGIC64 rounds to multiples of 64,
  per-partition AP scalars carry the bias) and the Sin ACT applies 2pi/64.
- Features: e-tile 0 bf16 (high-variance, near-exact); e-tiles 1-7 fp8
  stored (c,s)-interleaved so one DoubleRow contracts cos AND sin of an
  e-tile; K-side e-tiles 1-7 mean-centered (analytic Gaussian center,
  fp8-rounded) -- the dropped center terms are row-constant hence
  softmax-invariant; Q-side left uncentered (same reason).
- Scores: bf16 matmuls for e-tile 0 + 7 fp8 DoubleRows for e-tiles 1-7.
- exp/V/rowsum/AV: bf16 (consistent normalization), AV per key-block into
  PSUM with bf16 accumulators; production, scores and AV are interleaved
  in one pipeline so PE stays dense, and the final block fuses
  normalize+bias+store into its AV drain to shorten the tail.

kernel(**inputs) takes the full unsharded inputs from reference.setup_inputs()
and returns the full [B, S, D] output.
"""
import sys, math

sys.path.insert(0, "/opt/trn_rl_repo")

import numpy as np
import ml_dtypes

B, S, D = 2, 4096, 1024
NCORES = 8
QBLK = S // 4          # queries per core
ET = D // 128          # number of 128-row e/d tiles (8)
NF8 = ET - 1           # fp8 e-tiles (et1-7)
WSC = 64.0             # fp8 weight scale
MAGIC64 = float(1.5 * 2**23 * WSC)
TWOPI = 2.0 * math.pi
INV_SQRT_D = 1.0 / math.sqrt(D)
F8NP = ml_dtypes.float8_e4m3
XW = 1024              # x-block / feature-tile width

_cache = {}


def _build_program(s_keys=S, s_q=QBLK, trace_sim=False):
    import concourse.bass as bass
    from concourse import bacc
    import concourse.mybir as mybir
    import concourse.tile as tile
    from contextlib import ExitStack

    f32 = mybir.dt.float32
    f32r = mybir.dt.float32r
    f8 = mybir.dt.float8e4
    bf16 = mybir.dt.bfloat16
    Act = mybir.ActivationFunctionType
    Alu = mybir.AluOpType
    DR = mybir.MatmulPerfMode.DoubleRow

    n_kxb = s_keys // XW         # K x-blocks (1024 keys each)
    n_qxb = s_q // XW            # Q x-blocks
    n_tt = s_keys // 128         # key tiles
    NS = s_q                     # resident query width
    n_ns = NS // 512

    nc = bacc.Bacc("TRN2", target_bir_lowering=False, debug=False)

    xT = nc.dram_tensor("xT", [D, s_keys], f32, kind="ExternalInput").ap()
    xTq = nc.dram_tensor("xTq", [D, s_q], f32, kind="ExternalInput").ap()
    WH = {}
    WL = {}
    for w in ("q", "k", "v"):
        WH[w] = nc.dram_tensor(f"W{w}H", [D, D], f8, kind="ExternalInput").ap()
        WL[w] = nc.dram_tensor(f"W{w}L", [D, D], f8, kind="ExternalInput").ap()
    WQ0 = nc.dram_tensor("WQ0R", [D, 128], f32r, kind="ExternalInput").ap()
    WK0 = nc.dram_tensor("WK0R", [D, 128], f32r, kind="ExternalInput").ap()
    # CON columns: bqM x8 | bq64 x8 | bkM x8 | bk64 x8 | bv x8 | -ckm(c,s) x12
    NCON = 5 * ET + 2 * NF8
    CON = nc.dram_tensor("CON", [128, NCON], f32, kind="ExternalInput").ap()

    OT = nc.dram_tensor("OT", [D, s_q], f32, kind="ExternalOutput").ap()

    with tile.TileContext(nc, trace_sim=trace_sim) as tc, ExitStack() as top:
        dram = top.enter_context(tc.tile_pool(name="dram", bufs=1, space="DRAM"))
        # et0 cos/sin f32r; et1-7 fp8 interleaved (c1,s1,c2,s2,...) for the
        # DoubleRow (cos,sin) pairing
        CKa_d = [dram.tile([128, XW], bf16, tag=f"cka{i}", name=f"ckad{i}")
                 for i in range(n_kxb)]
        SKa_d = [dram.tile([128, XW], bf16, tag=f"ska{i}", name=f"skad{i}")
                 for i in range(n_kxb)]
        KB_d = [dram.tile([2 * NF8 * 128, XW], f8, tag=f"kb{i}", name=f"kbd{i}")
                for i in range(n_kxb)]
        V_d = [dram.tile([XW, D], bf16, tag=f"v{i}", name=f"vd{i}")
               for i in range(n_kxb)]

        cpool = top.enter_context(tc.tile_pool(name="consts", bufs=1))
        ctile = cpool.tile([128, NCON], f32, tag="ctile")
        nc.sync.dma_start(ctile[:], CON[:])
        bqM = [ctile[:, i : i + 1] for i in range(ET)]
        bq64 = [ctile[:, ET + i : ET + i + 1] for i in range(ET)]
        bkM = [ctile[:, 2 * ET + i : 2 * ET + i + 1] for i in range(ET)]
        bk64 = [ctile[:, 3 * ET + i : 3 * ET + i + 1] for i in range(ET)]
        bvt = [ctile[:, 4 * ET + i : 4 * ET + i + 1] for i in range(ET)]
        nck = [ctile[:, 5 * ET + i : 5 * ET + i + 1] for i in range(2 * NF8)]
        ones_bf = cpool.tile([128, 2], bf16, tag="ones_bf")
        nc.vector.memset(ones_bf[:], 1.0)
        ones_rf = cpool.tile([1, 128], f32, tag="ones_rf")
        nc.vector.memset(ones_rf[:], 1.0)
        ones_row = cpool.tile([1, 128], f32r, tag="ones_row")
        nc.vector.tensor_copy(ones_row[:], ones_rf[:])

        # resident Q feature maps: et0 f32r; et1-7 fp8 interleaved (c,s) pairs
        qres = top.enter_context(tc.tile_pool(name="qres", bufs=1))
        cqr = qres.tile([128, NS], bf16, tag="cqr")
        sqr = qres.tile([128, NS], bf16, tag="sqr")
        qf8 = qres.tile([128, 2 * NF8 * NS], f8, tag="qf8")

        wpool_ctx = tc.tile_pool(name="w", bufs=1)
        wpool = wpool_ctx.__enter__()
        wa_h = [wpool.tile([128, 2 * D], f8, tag=f"wah{j}", name=f"wah{j}") for j in range(4)]
        wa_l = [wpool.tile([128, 2 * D], f8, tag=f"wal{j}", name=f"wal{j}") for j in range(4)]
        wb_h = [wpool.tile([128, 2 * D], f8, tag=f"wbh{j}", name=f"wbh{j}") for j in range(4)]
        wb_l = [wpool.tile([128, 2 * D], f8, tag=f"wbl{j}", name=f"wbl{j}") for j in range(4)]
        w0r = [wpool.tile([128, 128], f32r, tag=f"w0r{d}", name=f"w0r{d}") for d in range(ET)]

        def load_w(tiles, src):
            for j in range(4):
                nc.sync.dma_start(
                    tiles[j][:].rearrange("p (two e) -> p two e", two=2),
                    src[2 * j * 128 : (2 * j + 2) * 128, :]
                    .rearrange("(two p) e -> p two e", p=128))

        def load_w0(tiles, src):
            for dd in range(ET):
                nc.sync.dma_start(tiles[dd][:], src[dd * 128 : (dd + 1) * 128, :])

        # ============ merged: projections + features + scores + AV ============
        with tc.tile_pool(name="p1sb", bufs=2) as p1, \
             tc.tile_pool(name="p1chain", bufs=2) as pch, \
             tc.tile_pool(name="p2sb", bufs=2) as p2, \
             tc.tile_pool(name="epool", bufs=9) as epool, \
             tc.tile_pool(name="vpool", bufs=9) as vpool, \
             tc.tile_pool(name="oacc", bufs=1) as oacc, \
             tc.tile_pool(name="psum1", bufs=1, space="PSUM") as ps1:

            def load_xblk(src_ap, col0):
                src3 = src_ap[:, col0 : col0 + XW].rearrange("(d p) s -> p d s", p=128)
                xr = p1.tile([128, ET * XW], f32r, tag="xbr", name="xbr", bufs=1)
                nc.gpsimd.dma_start(xr[:].rearrange("p (d s) -> p d s", d=ET), src3)
                xh = p1.tile([128, ET * XW], f8, tag="xh8", name="xh8", bufs=1)
                nc.gpsimd.dma_start(xh[:].rearrange("p (d s) -> p d s", d=ET), src3)
                xl = p1.tile([128, ET * XW], f8, tag="xl8", name="xl8", bufs=1)
                # split the residual op so early d-pairs unblock sooner
                half = ET * XW // 2
                nc.vector.tensor_tensor(xl[:, :half], xr[:, :half], xh[:, :half],
                                        Alu.subtract)
                nc.vector.tensor_tensor(xl[:, half:], xr[:, half:], xh[:, half:],
                                        Alu.subtract)
                return xh, xl, xr

            def feature_block(xblk, et, w_h, w_l, w0, bM, b64, ncen, c_dst, s_dst,
                              center):
                """cos/sin feature tiles [128, XW] for one e-tile."""
                xh, xl, xr = xblk
                xh3 = xh[:].rearrange("p (d s) -> p d s", d=ET)
                xl3 = xl[:].rearrange("p (d s) -> p d s", d=ET)
                esl = slice(et * 128, (et + 1) * 128)
                ps = ps1.tile([128, XW], f32, tag="proj", name="psf", bufs=2)
                for h in range(XW // 512):
                    csl = slice(h * 512, (h + 1) * 512)
                    if et == 0:
                        for dd in range(ET):
                            nc.tensor.matmul(
                                ps[:, csl], w0[dd][:],
                                xr[:, dd * XW + h * 512 : dd * XW + (h + 1) * 512],
                                start=(dd == 0), stop=(dd == ET - 1))
                    else:
                        # et1: 3-pass; et2-7: 2-pass (inv_wl shrinks the
                        # W-residual term below the feature noise floor)
                        passes = ((xh3, w_h), (xl3, w_h), (xh3, w_l))
                        if et >= 2:
                            passes = passes[:2]
                        for pi, (xs, ws) in enumerate(passes):
                            for j in range(4):
                                lhsT = (ws[j][:].rearrange("p (two e) -> p two e",
                                                           two=2)[:, :, esl])
                                rhs = xs[:, 2 * j : 2 * j + 2, csl]
                                nc.tensor.matmul(
                                    ps[:, csl], lhsT, rhs,
                                    start=(pi == 0 and j == 0),
                                    stop=(pi == len(passes) - 1 and j == 3),
                                    perf_mode=DR)
                # turns-space range reduction at the x64 scale
                kk = pch.tile([128, XW], f32, tag="kk", name="kk")
                nc.vector.tensor_scalar(kk[:], ps[:], bM[et][:], MAGIC64,
                                        Alu.add, Alu.subtract)
                f = pch.tile([128, XW], f32, tag="f", name="f")
                nc.vector.scalar_tensor_tensor(f[:], ps[:], b64[et][:], kk[:],
                                               Alu.add, Alu.subtract)
                if center and et >= 1:
                    s32 = pch.tile([128, XW], f32, tag="s32", name="s32", bufs=1)
                    nc.scalar.activation(s32[:], f[:], Act.Sin, scale=TWOPI / WSC)
                    nc.scalar.activation(s_dst, s32[:], Act.Identity,
                                         bias=ncen[NF8 + et - 1][:])
                else:
                    nc.scalar.activation(s_dst, f[:], Act.Sin, scale=TWOPI / WSC)
                g = pch.tile([128, XW], f32, tag="kk", name="g")
                nc.vector.add_range_wrap(g[:], f[:], 0.25 * WSC, 0.5 * WSC,
                                         1.0 * WSC)
                if center and et >= 1:
                    c32 = pch.tile([128, XW], f32, tag="s32", name="c32", bufs=1)
                    nc.scalar.activation(c32[:], g[:], Act.Sin, scale=TWOPI / WSC)
                    nc.scalar.activation(c_dst, c32[:], Act.Identity,
                                         bias=ncen[et - 1][:])
                else:
                    nc.scalar.activation(c_dst, g[:], Act.Sin, scale=TWOPI / WSC)

            # --- Q features; K weights prefetch into the dedicated set ---
            load_w0(w0r, WQ0)
            load_w(wa_h, WH["q"])
            load_w(wa_l, WL["q"])
            load_w(wb_h, WH["k"])
            load_w(wb_l, WL["k"])
            for qxb in range(n_qxb):
                xqb = load_xblk(xTq, qxb * XW)
                for et in range(ET):
                    if et < 1:
                        cd = cqr[:, qxb * XW : qxb * XW + XW]
                        sd = sqr[:, qxb * XW : qxb * XW + XW]
                    else:
                        e8 = et - 1
                        cd = qf8[:, (2 * e8) * NS + qxb * XW :
                                 (2 * e8) * NS + qxb * XW + XW]
                        sd = qf8[:, (2 * e8 + 1) * NS + qxb * XW :
                                 (2 * e8 + 1) * NS + qxb * XW + XW]
                    feature_block(xqb, et, wa_h, wa_l, w0r, bqM, bq64, None,
                                  cd, sd, center=False)

            # V weights overwrite the q slots; K et0 weights overwrite w0r
            load_w(wa_h, WH["v"])
            load_w(wa_l, WL["v"])
            load_w0(w0r, WK0)

            ps_rs = ps1.tile([2, NS], f32, tag="rs", bufs=1)
            qf83 = qf8[:].rearrange("p (e n) -> p e n", e=2 * NF8)
            o_ac = [oacc.tile([128, NS], bf16, tag=f"o{dt}", name=f"oac{dt}")
                    for dt in range(ET)]

            # --- per K-block: features (centered et1-7 fp8) + V (bf16)
            #     + scores/exp/rowsum + group AV, all interleaved ---
            for kxb in range(n_kxb):
                xkb = load_xblk(xT, kxb * XW)
                for et in range(ET):
                    if et < 1:
                        cst = pch.tile([128, XW], bf16, tag="cst", name="cst", bufs=1)
                        sst = pch.tile([128, XW], bf16, tag="sst", name="sst", bufs=1)
                        feature_block(xkb, et, wb_h, wb_l, w0r, bkM, bk64,
                                      nck, cst[:], sst[:], center=True)
                        nc.sync.dma_start(CKa_d[kxb][:, :], cst[:])
                        nc.sync.dma_start(SKa_d[kxb][:, :], sst[:])
                    else:
                        e8 = et - 1
                        cst = pch.tile([128, XW], f8, tag="cst8", name="cst8")
                        sst = pch.tile([128, XW], f8, tag="sst8", name="sst8")
                        feature_block(xkb, et, wb_h, wb_l, w0r, bkM, bk64,
                                      nck, cst[:], sst[:], center=True)
                        nc.sync.dma_start(
                            KB_d[kxb][(2 * e8) * 128 : (2 * e8 + 1) * 128, :],
                            cst[:])
                        nc.sync.dma_start(
                            KB_d[kxb][(2 * e8 + 1) * 128 : (2 * e8 + 2) * 128, :],
                            sst[:])
                # V
                xh, xl, _xr = xkb
                xh3 = xh[:].rearrange("p (d s) -> p d s", d=ET)
                xl3 = xl[:].rearrange("p (d s) -> p d s", d=ET)
                for ti in range(XW // 128):
                    tsl = slice(ti * 128, (ti + 1) * 128)
                    for dg in range(2):
                        psv = ps1.tile([128, 512], f32, tag="pv", name="psv",
                                       bufs=2)
                        first = True
                        for xs, ws in ((xh3, wa_h), (xl3, wa_h), (xh3, wa_l)):
                            for j in range(4):
                                lhsT = xs[:, 2 * j : 2 * j + 2, tsl]
                                rhs = (ws[j][:].rearrange("p (two e) -> p two e",
                                                          two=2)
                                       [:, :, dg * 512 : dg * 512 + 512])
                                last = (xs is xh3 and ws is wa_l and j == 3)
                                nc.tensor.matmul(psv[:], lhsT, rhs,
                                                 start=first, stop=last,
                                                 perf_mode=DR)
                                first = False
                        vstg = p1.tile([128, 512], bf16, tag="vstg", name="vstg", bufs=1)
                        nc.vector.tensor_scalar(vstg[:], psv[:], 1.0 / WSC, None,
                                                Alu.mult)
                        nc.sync.dma_start(
                            V_d[kxb][ti * 128 : (ti + 1) * 128,
                                     dg * 512 : (dg + 1) * 512], vstg[:])

                # --- scores + exp + rowsum for this block's 8 t-tiles ---
                e_tiles = []
                for loc in range(XW // 128):
                    tt = kxb * (XW // 128) + loc
                    cka = p2.tile([128, 128], bf16, tag="cka", name="cka")
                    ska = p2.tile([128, 128], bf16, tag="ska", name="ska")
                    kb = p2.tile([128, 2 * NF8 * 128], f8, tag="kb", name="kb")
                    nc.sync.dma_start(cka[:],
                                      CKa_d[kxb][:, loc * 128 : (loc + 1) * 128])
                    nc.sync.dma_start(ska[:],
                                      SKa_d[kxb][:, loc * 128 : (loc + 1) * 128])
                    nc.sync.dma_start(
                        kb[:].rearrange("p (e t) -> p e t", e=2 * NF8),
                        KB_d[kxb][:, loc * 128 : (loc + 1) * 128]
                        .rearrange("(e p) t -> p e t", p=128))
                    kb3 = kb[:].rearrange("p (e t) -> p e t", e=2 * NF8)
                    ps_sim = ps1.tile([128, NS], f32, tag="proj", name="ps_sim",
                                      bufs=2)
                    for ns in range(n_ns):
                        sl = slice(ns * 512, ns * 512 + 512)
                        nc.tensor.matmul(ps_sim[:, sl], cka[:],
                                         cqr[:, ns * 512 : ns * 512 + 512],
                                         start=True, stop=False)
                        nc.tensor.matmul(ps_sim[:, sl], ska[:],
                                         sqr[:, ns * 512 : ns * 512 + 512],
                                         start=False, stop=False)
                        for pr in range(NF8):
                            nc.tensor.matmul(ps_sim[:, sl],
                                             kb3[:, 2 * pr : 2 * pr + 2, :],
                                             qf83[:, 2 * pr : 2 * pr + 2, sl],
                                             start=False,
                                             stop=(pr == NF8 - 1), perf_mode=DR)
                    et_t = epool.tile([128, NS], bf16, tag="e", name="e")
                    nc.scalar.activation(et_t[:], ps_sim[:], Act.Exp,
                                         scale=INV_SQRT_D)
                    e_tiles.append(et_t)
                    for ns in range(n_ns):
                        sl = slice(ns * 512, ns * 512 + 512)
                        nc.tensor.matmul(ps_rs[:, sl], ones_bf[:], et_t[:, sl],
                                         start=(tt == 0), stop=(tt == n_tt - 1))

                if kxb == n_kxb - 1:
                    # rowsum complete: reciprocal + broadcast now so the
                    # final normalize can fuse into this block's AV drain
                    rs_sb = p2.tile([1, NS], f32, tag="rs_sb", bufs=1)
                    nc.vector.tensor_copy(rs_sb[:], ps_rs[:1, :])
                    rec_f = p2.tile([1, NS], f32, tag="rec_f", bufs=1)
                    nc.vector.reciprocal(rec_f[:], rs_sb[:])
                    rec = p2.tile([1, NS], f32r, tag="rec", bufs=1)
                    nc.vector.tensor_copy(rec[:], rec_f[:])
                    bc = p2.tile([128, NS], f32, tag="bc", bufs=1)
                    for ns in range(n_ns):
                        sl = slice(ns * 512, ns * 512 + 512)
                        ps_bc = ps1.tile([128, 512], f32, tag="pv",
                                         name="ps_bc", bufs=2)
                        nc.tensor.matmul(ps_bc[:], ones_row[:], rec[:, sl],
                                         start=True, stop=True)
                        nc.vector.tensor_copy(bc[:, sl], ps_bc[:])

                # --- group AV for this block (accumulate into o_ac) ---
                for dg in range(2):
                    vts = []
                    for loc in range(XW // 128):
                        vt = vpool.tile([128, 512], bf16, tag="vt", name="vt")
                        nc.sync.dma_start(
                            vt[:], V_d[kxb][loc * 128 : (loc + 1) * 128,
                                            dg * 512 : (dg + 1) * 512])
                        vts.append(vt)
                    for di in range(4):
                        dt = dg * 4 + di
                        ps_o = ps1.tile([128, NS], f32, tag="proj", name="ps_o",
                                        bufs=2)
                        for loc in range(XW // 128):
                            for ns in range(n_ns):
                                sl = slice(ns * 512, ns * 512 + 512)
                                nc.tensor.matmul(
                                    ps_o[:, sl],
                                    vts[loc][:, di * 128 : (di + 1) * 128],
                                    e_tiles[loc][:, sl],
                                    start=(loc == 0),
                                    stop=(loc == XW // 128 - 1))
                        if kxb == 0:
                            nc.vector.tensor_copy(o_ac[dt][:], ps_o[:])
                        elif kxb < n_kxb - 1:
                            nc.vector.tensor_tensor(o_ac[dt][:], ps_o[:],
                                                    o_ac[dt][:], Alu.add)
                        else:
                            # final block: (ps_o + o_ac) * bc + bv -> OT
                            on = p2.tile([128, NS], f32, tag="on", name="on")
                            nc.vector.tensor_tensor(on[:], ps_o[:],
                                                    o_ac[dt][:], Alu.add)
                            nc.vector.tensor_tensor(on[:], on[:], bc[:],
                                                    Alu.mult)
                            nc.scalar.activation(on[:], on[:], Act.Identity,
                                                 bias=bvt[dt][:])
                            nc.sync.dma_start(OT[dt * 128 : (dt + 1) * 128, :],
                                              on[:])

        wpool_ctx.__exit__(None, None, None)

    nc.compile()
    return nc


def _f8(a):
    return np.asarray(a, np.float32).astype(F8NP)


def _host_prep(x, Wq, bq, Wk, bk, Wv, bv, phase_bias):
    wavelengths = np.arange(1, D + 1, dtype=np.float32) * np.float32(2.0 * math.pi / D)
    inv_wl = (np.float32(1.0) / (wavelengths + np.float32(1e-8))).astype(np.float32)
    turn_sc = (inv_wl / np.float32(TWOPI)).astype(np.float32)   # per-feature
    bq2 = ((bq * inv_wl + phase_bias) / TWOPI).astype(np.float32)
    bk2 = ((bk * inv_wl + phase_bias) / TWOPI).astype(np.float32)

    W8 = {}
    W0R = {}
    for nm, W in (("q", Wq), ("k", Wk), ("v", Wv)):
        WT = np.ascontiguousarray(W.T).astype(np.float32)
        if nm != "v":
            WT = WT * turn_sc[None, :]      # fold inv_wl/2pi per feature col
            W0R[nm] = np.ascontiguousarray(WT[:, :128] * np.float32(WSC))
        WTs = WT * np.float32(WSC)
        wh = _f8(WTs)
        wl = _f8(WTs - wh.astype(np.float32))
        W8[nm] = (wh, wl)

    def centers(W, bias):
        mu = (bias * inv_wl + phase_bias).astype(np.float64)
        sg = np.sqrt((W.astype(np.float64) ** 2).sum(1)) * inv_wl
        att = np.exp(-(sg ** 2) / 2)
        cc = (np.cos(mu) * att).astype(np.float32)
        ss = (np.sin(mu) * att).astype(np.float32)
        return _f8(cc).astype(np.float32), _f8(ss).astype(np.float32)

    ckm_c, ckm_s = centers(Wk, bk)

    # CON: bqM | bq64 | bkM | bk64 | bv | -ckm(c et2-7, s et2-7)
    cols = [
        (np.float32(WSC) * bq2 + np.float32(MAGIC64)).reshape(ET, 128),
        (np.float32(WSC) * bq2).reshape(ET, 128),
        (np.float32(WSC) * bk2 + np.float32(MAGIC64)).reshape(ET, 128),
        (np.float32(WSC) * bk2).reshape(ET, 128),
        bv.reshape(ET, 128).astype(np.float32),
        (-ckm_c.reshape(ET, 128)[1:]),
        (-ckm_s.reshape(ET, 128)[1:]),
    ]
    con = np.concatenate(cols, axis=0)
    con = np.ascontiguousarray(con.T).astype(np.float32)

    xT = [np.ascontiguousarray(x[b].T).astype(np.float32) for b in range(x.shape[0])]
    return xT, W8, W0R, con


def kernel(x, Wq, bq, Wk, bk, Wv, bv, phase_bias, _trace=False):
    from concourse.bass_utils import run_bass_kernel_spmd

    x = np.asarray(x, dtype=np.float32)
    xT, W8, W0R, con = _host_prep(
        x, np.asarray(Wq, np.float32), np.asarray(bq, np.float32),
        np.asarray(Wk, np.float32), np.asarray(bk, np.float32),
        np.asarray(Wv, np.float32), np.asarray(bv, np.float32),
        np.asarray(phase_bias, np.float32))

    if "prog" not in _cache:
        _cache["prog"] = _build_program()
    nc = _cache["prog"]

    in_maps = []
    for c in range(NCORES):
        b, qb = c // 4, c % 4
        in_maps.append({
            "xT": xT[b],
            "xTq": np.ascontiguousarray(xT[b][:, qb * QBLK : (qb + 1) * QBLK]),
            "WqH": W8["q"][0], "WqL": W8["q"][1],
            "WkH": W8["k"][0], "WkL": W8["k"][1],
            "WvH": W8["v"][0], "WvL": W8["v"][1],
            "WQ0R": W0R["q"], "WK0R": W0R["k"],
            "CON": con,
        })
    res = run_bass_kernel_spmd(nc, in_maps, core_ids=list(range(NCORES)),
                               trace=_trace)
    out = np.empty((B, S, D), dtype=np.float32)
    for c in range(NCORES):
        b, qb = c // 4, c % 4
        out[b, qb * QBLK : (qb + 1) * QBLK, :] = res.results[c]["OT"].T
    if _trace:
        kernel.last_exec_time_ns = res.exec_time_ns
        kernel.last_result = res
    return out
